# revision 32
# baseline (speedup 1.0000x reference)
"""Trainium2 Bass kernel: inclusive cumsum along L for X (4, 8192, 32, 32) f32.

Strategy (8 NeuronCores, SPMD), v7 "pipelined all-matmul fp16":
  - View X as (B=4, L=8192, C=1024), shard 8 ways: core i gets b = i//2,
    c-half h = i%2 -> a (8192, 512) slab, converted to fp16 on the host
    (error budget is 2e-2 of max|cumsum|~400; fp16 gives ~1e-3). HBM
    traffic per core: 8.4 MB in + 8.4 MB out ~ 47 us at the ~358 GB/s
    per-core limit. The host pre-swizzles each slab to (g, p, ks, c)
    order so every DMA is a fully contiguous block, and un-swizzles the
    output.
  - Compute is all TensorE matmuls in natural (l-partition, c-free)
    layout (back-to-back matmuls stream at ~216 ns for N=512), pipelined
    in 8 groups of 8 l-blocks (block = 128 l):
      group g: 512 KiB in-DMAs (sync HWDGE ring, FIFO order) ->
      8 suffix-mask matmuls (sliding-window constant; lhsT[k,m]=[m>t])
      accumulate THE CARRIES directly into a persistent [64,512] PSUM
      bank (row m = sum of totals of blocks < m) -> one ACT full-bank
      fp16 copy -> SWDGE accumulate-DMA adds carry_t into l-row 0 of
      block t; next iteration: 8 back-to-back UT-ones matmuls
      (cumsum = UT @ [x0+carry; x1; ...]) -> ACT/DVE PSUM->SBUF fp16
      copies -> 512 KiB out-DMAs (gpsimd).
"""

import numpy as np
from contextlib import ExitStack

import concourse.bass as bass
import concourse.tile as tile
from concourse import bacc, mybir
from concourse.bass_utils import run_bass_kernel_spmd

N_CORES = 8
B, L, D, N = 4, 8192, 32, 32
C_FULL = D * N          # 1024 columns per batch entry
C = C_FULL // 2         # 512 columns per core
P = 128                 # partitions / l-block size
NBLK = L // P           # 64 l-blocks per core
KS = 8                  # l-blocks per group
NG = NBLK // KS         # 8 groups
CW = P + 2 * NBLK       # const pack width (ut 128 | suffix window 127 | pad)

_CACHE = {}


def _host_consts():
    """One [128, 256] fp16 pack: cols 0-127 = UT ones (k<=m); cols
    128-254 = sliding suffix window W[k, c] = [c > 63] so that
    W[:, 63-t : 127-t] has ones exactly in columns m > t."""
    cst = np.zeros((P, CW), dtype=np.float16)
    cst[:, :P] = np.triu(np.ones((P, P), dtype=np.float16))
    for c in range(P, P + 2 * NBLK - 1):
        if (c - P) > NBLK - 1:
            cst[:, c] = 1.0
    return cst


def _build_program():
    f16 = mybir.dt.float16
    f32 = mybir.dt.float32
    nc = bacc.Bacc(
        trn_type="TRN2", debug=False, num_devices=N_CORES, num_swdge_queues=2
    )
    x = nc.dram_tensor("x", [NG, P, KS * C], f16, kind="ExternalInput").ap()
    y = nc.dram_tensor("y", [NG, P, KS * C], f16, kind="ExternalOutput").ap()
    cstd = nc.dram_tensor("cst", [P, CW], f16, kind="ExternalInput").ap()

    with tile.TileContext(nc) as tc, ExitStack() as ctx:
        const_pool = ctx.enter_context(tc.tile_pool(name="const", bufs=1))
        x_pool = ctx.enter_context(tc.tile_pool(name="xin", bufs=1))
        y_pool = ctx.enter_context(tc.tile_pool(name="yout", bufs=4))
        tot_psum = ctx.enter_context(tc.tile_pool(name="totp", bufs=1, space="PSUM"))
        main_psum = ctx.enter_context(tc.tile_pool(name="mainp", bufs=3, space="PSUM"))

        cst_sb = const_pool.tile([P, CW], f16, name="cst")
        # ping-pong carry snapshots: copy_{g+1} must not WAR-wait on
        # accum_g reading the previous snapshot
        car_sb = [
            const_pool.tile([NBLK, C], f16, name=f"car{i}") for i in range(2)
        ]
        xall = x_pool.tile([P, NBLK * C], f16, name="xall")
        ut_sb = cst_sb[:, :P]

        # consts ride gpsimd's SWDGE ring (free early) so the sync ring
        # is a pure input stream from the first instruction
        nc.gpsimd.dma_start(out=cst_sb[:], in_=cstd[:, :])

        # HAM warmup: ~4.6us of junk matmuls on memset weights while the
        # first input DMA is in flight, so the PE clock is at 2.4 GHz
        # (K=8/8) when real work starts.
        wjunk = const_pool.tile([P, P], f16, name="wjunk")
        nc.gpsimd.memset(wjunk[:], 0.0)
        wbank = tot_psum.tile([P, P], f32, name="wbank", tag="wb", bufs=1)
        for w in range(16):
            nc.tensor.matmul(
                wbank[:], wjunk[:], wjunk[:], start=True, stop=True
            )

        # All in-DMAs up front on the sync HWDGE ring: FIFO order means
        # group g's data lands before group g+1's, so compute pipelines.
        for g in range(NG):
            xg = xall[:, g * KS * C : (g + 1) * KS * C]
            nc.sync.dma_start(out=xg, in_=x[g])

        totP = tot_psum.tile([NBLK, C], f32, name="totP")

        def carry_stage(g):
            """Suffix-mask matmuls accumulate block carries; fold into
            l-row 0 of each block of group g via accumulate-DMA."""
            for j in range(KS):
                t = g * KS + j
                lo = P + NBLK - 1 - t
                nc.tensor.matmul(
                    totP[:],
                    cst_sb[:, lo : lo + NBLK],
                    xall[:, t * C : (t + 1) * C],
                    start=(t == 0),
                    stop=(t == NBLK - 1),
                )
            # rows m <= 8g+8 are final after this group's matmuls; the
            # accum-DMA only reads rows 8g..8g+7.
            car = car_sb[g % 2]
            nc.scalar.copy(car[:], totP[:])
            dst = xall[0:1, g * KS * C : (g + 1) * KS * C].rearrange(
                "p (j c) -> p j c", j=KS
            )
            nc.gpsimd.dma_start(
                out=dst,
                in_=car[g * KS : (g + 1) * KS, :],
                accum_op=mybir.AluOpType.add,
            )

        def ut_stage(g):
            """8 back-to-back UT matmuls into 2-bank PSUM tiles; one big
            copy per tile pair (ACT: pairs 0-1 + carry copy; DVE: 2-3);
            stream halves to HBM from gpsimd."""
            yb = y_pool.tile([P, KS * C], f16, name="yb", tag="yb", bufs=4)
            H = KS // 2
            for pair in range(KS // 2):
                bank = main_psum.tile([P, 2 * C], f32, name="bank", tag="bank", bufs=3)
                for s in range(2):
                    j = pair * 2 + s
                    t = g * KS + j
                    nc.tensor.matmul(
                        bank[:, s * C : (s + 1) * C],
                        ut_sb,
                        xall[:, t * C : (t + 1) * C],
                        start=True,
                        stop=True,
                    )
                dst = yb[:, pair * 2 * C : (pair + 1) * 2 * C]
                if pair < 2:
                    nc.scalar.copy(dst, bank[:])
                else:
                    nc.vector.tensor_copy(dst, bank[:])
                if pair == 3:
                    if g == NG - 1:
                        # last group: halves so the final DMA tail is short
                        yh = y[g].rearrange("p (h c) -> p h c", h=2)
                        nc.gpsimd.dma_start(out=yh[:, 0, :], in_=yb[:, : H * C])
                        nc.gpsimd.dma_start(out=yh[:, 1, :], in_=yb[:, H * C :])
                    else:
                        nc.gpsimd.dma_start(out=y[g], in_=yb[:])

        # software pipeline, 3-iteration skew: carries(it) | UT(it-3) —
        # the carry chain (~5.5us) fits in the FIFO slack.
        SKEW = 3
        for it in range(NG + SKEW):
            if it < NG:
                carry_stage(it)
            if it >= SKEW:
                ut_stage(it - SKEW)

    nc.compile()
    return nc


def _get_program():
    if "nc" not in _CACHE:
        _CACHE["nc"] = _build_program()
    return _CACHE["nc"]


def _shard(X):
    """(4, 8192, 32, 32) f32 -> 8 fp16 slabs swizzled to (g, p, ks, c)."""
    Xv = X.reshape(B, L, C_FULL)
    shards = []
    for i in range(N_CORES):
        b, h = i // 2, i % 2
        s = Xv[b, :, h * C : (h + 1) * C].astype(np.float16)    # (8192, 512)
        s = s.reshape(NG, KS, P, C).transpose(0, 2, 1, 3)       # (g, p, ks, c)
        shards.append(np.ascontiguousarray(s.reshape(NG, P, KS * C)))
    return shards


def _unshard(parts):
    out = np.empty((B, L, C_FULL), dtype=np.float32)
    for i in range(N_CORES):
        b, h = i // 2, i % 2
        p = parts[i].reshape(NG, P, KS, C).transpose(0, 2, 1, 3)  # (g, ks, p, c)
        out[b, :, h * C : (h + 1) * C] = p.reshape(L, C).astype(np.float32)
    return out.reshape(B, L, D, N)


def kernel(X_in, _trace=False, _tmpdir=None, _trace_cores=None):
    X = np.asarray(X_in, dtype=np.float32)
    assert X.shape == (B, L, D, N), X.shape
    nc = _get_program()
    cst = _host_consts()
    in_maps = [{"x": s, "cst": cst} for s in _shard(X)]
    kwargs = {}
    if _trace:
        kwargs = dict(
            trace=True,
            tmpdir=_tmpdir,
            trace_cores=_trace_cores or list(range(N_CORES)),
        )
    res = run_bass_kernel_spmd(nc, in_maps, core_ids=list(range(N_CORES)), **kwargs)
    out = _unshard([res.results[i]["y"] for i in range(N_CORES)])
    kernel.last_results = res
    return out


# revision 34
# speedup vs baseline: 1.0477x; 1.0477x over previous
"""Trainium2 Bass kernel: inclusive cumsum along L for X (4, 8192, 32, 32) f32.

Strategy (8 NeuronCores, SPMD), v7 "pipelined all-matmul fp16":
  - View X as (B=4, L=8192, C=1024), shard 8 ways: core i gets b = i//2,
    c-half h = i%2 -> a (8192, 512) slab, converted to fp16 on the host
    (error budget is 2e-2 of max|cumsum|~400; fp16 gives ~1e-3). HBM
    traffic per core: 8.4 MB in + 8.4 MB out ~ 47 us at the ~358 GB/s
    per-core limit. The host pre-swizzles each slab to (g, p, ks, c)
    order so every DMA is a fully contiguous block, and un-swizzles the
    output.
  - Compute is all TensorE matmuls in natural (l-partition, c-free)
    layout (back-to-back matmuls stream at ~216 ns for N=512), pipelined
    in 8 groups of 8 l-blocks (block = 128 l):
      group g: 512 KiB in-DMAs (sync HWDGE ring, FIFO order) ->
      8 suffix-mask matmuls (sliding-window constant; lhsT[k,m]=[m>t])
      accumulate THE CARRIES directly into a persistent [64,512] PSUM
      bank (row m = sum of totals of blocks < m) -> one ACT full-bank
      fp16 copy -> SWDGE accumulate-DMA adds carry_t into l-row 0 of
      block t; next iteration: 8 back-to-back UT-ones matmuls
      (cumsum = UT @ [x0+carry; x1; ...]) -> ACT/DVE PSUM->SBUF fp16
      copies -> 512 KiB out-DMAs (gpsimd).
"""

import numpy as np
from contextlib import ExitStack

import concourse.bass as bass
import concourse.tile as tile
from concourse import bacc, mybir
from concourse.bass_utils import run_bass_kernel_spmd

N_CORES = 8
B, L, D, N = 4, 8192, 32, 32
C_FULL = D * N          # 1024 columns per batch entry
C = C_FULL // 2         # 512 columns per core
P = 128                 # partitions / l-block size
NBLK = L // P           # 64 l-blocks per core
KS = 8                  # l-blocks per group
NG = NBLK // KS         # 8 groups
CW = P + 2 * NBLK       # const pack width (ut 128 | suffix window 127 | pad)

_CACHE = {}


def _host_consts():
    """One [128, 256] fp16 pack: cols 0-127 = UT ones (k<=m); cols
    128-254 = sliding suffix window W[k, c] = [c > 63] so that
    W[:, 63-t : 127-t] has ones exactly in columns m > t."""
    cst = np.zeros((P, CW), dtype=np.float16)
    cst[:, :P] = np.triu(np.ones((P, P), dtype=np.float16))
    for c in range(P, P + 2 * NBLK - 1):
        if (c - P) > NBLK - 1:
            cst[:, c] = 1.0
    return cst


def _build_program():
    f16 = mybir.dt.float16
    f32 = mybir.dt.float32
    nc = bacc.Bacc(
        trn_type="TRN2", debug=False, num_devices=N_CORES, num_swdge_queues=2
    )
    x = nc.dram_tensor("x", [NG, P, KS * C], f16, kind="ExternalInput").ap()
    y = nc.dram_tensor("y", [NG, P, KS * C], f16, kind="ExternalOutput").ap()
    cstd = nc.dram_tensor("cst", [P, CW], f16, kind="ExternalInput").ap()

    with tile.TileContext(nc) as tc, ExitStack() as ctx:
        const_pool = ctx.enter_context(tc.tile_pool(name="const", bufs=1))
        x_pool = ctx.enter_context(tc.tile_pool(name="xin", bufs=1))
        y_pool = ctx.enter_context(tc.tile_pool(name="yout", bufs=4))
        tot_psum = ctx.enter_context(tc.tile_pool(name="totp", bufs=1, space="PSUM"))
        main_psum = ctx.enter_context(tc.tile_pool(name="mainp", bufs=3, space="PSUM"))

        cst_sb = const_pool.tile([P, CW], f16, name="cst")
        # ping-pong carry snapshots: copy_{g+1} must not WAR-wait on
        # accum_g reading the previous snapshot
        car_sb = [
            const_pool.tile([NBLK, C], f16, name=f"car{i}") for i in range(2)
        ]
        xall = x_pool.tile([P, NBLK * C], f16, name="xall")
        ut_sb = cst_sb[:, :P]

        # consts ride gpsimd's SWDGE ring (free early) so the sync ring
        # is a pure input stream from the first instruction
        nc.gpsimd.dma_start(out=cst_sb[:], in_=cstd[:, :])

        # HAM warmup: ~4.6us of junk matmuls on memset weights while the
        # first input DMA is in flight, so the PE clock is at 2.4 GHz
        # (K=8/8) when real work starts.
        wjunk = const_pool.tile([P, P], f16, name="wjunk")
        nc.gpsimd.memset(wjunk[:], 0.0)
        wbank = tot_psum.tile([P, P], f32, name="wbank", tag="wb", bufs=1)
        for w in range(16):
            nc.tensor.matmul(
                wbank[:], wjunk[:], wjunk[:], start=True, stop=True
            )

        # All in-DMAs up front on the sync HWDGE ring: FIFO order means
        # group g's data lands before group g+1's, so compute pipelines.
        # 512 KiB halves so the first blocks land sooner.
        HC = KS * C // 2
        for g in range(NG):
            for h in range(2):
                xg = xall[:, g * KS * C + h * HC : g * KS * C + (h + 1) * HC]
                nc.sync.dma_start(out=xg, in_=x[g][:, h * HC : (h + 1) * HC])

        totP = tot_psum.tile([NBLK, C], f32, name="totP")

        def carry_stage(g):
            """Suffix-mask matmuls accumulate block carries; fold into
            l-row 0 of each block of group g via accumulate-DMA."""
            for j in range(KS):
                t = g * KS + j
                lo = P + NBLK - 1 - t
                nc.tensor.matmul(
                    totP[:],
                    cst_sb[:, lo : lo + NBLK],
                    xall[:, t * C : (t + 1) * C],
                    start=(t == 0),
                    stop=(t == NBLK - 1),
                )
            # rows m <= 8g+8 are final after this group's matmuls; the
            # accum-DMA only reads rows 8g..8g+7.
            car = car_sb[g % 2]
            nc.scalar.copy(car[:], totP[:])
            dst = xall[0:1, g * KS * C : (g + 1) * KS * C].rearrange(
                "p (j c) -> p j c", j=KS
            )
            nc.gpsimd.dma_start(
                out=dst,
                in_=car[g * KS : (g + 1) * KS, :],
                accum_op=mybir.AluOpType.add,
            )

        def ut_stage(g):
            """8 back-to-back UT matmuls into 2-bank PSUM tiles; one big
            copy per tile pair (ACT: pairs 0-1 + carry copy; DVE: 2-3);
            stream halves to HBM from gpsimd."""
            yb = y_pool.tile([P, KS * C], f16, name="yb", tag="yb", bufs=4)
            H = KS // 2
            for pair in range(KS // 2):
                bank = main_psum.tile([P, 2 * C], f32, name="bank", tag="bank", bufs=3)
                for s in range(2):
                    j = pair * 2 + s
                    t = g * KS + j
                    nc.tensor.matmul(
                        bank[:, s * C : (s + 1) * C],
                        ut_sb,
                        xall[:, t * C : (t + 1) * C],
                        start=True,
                        stop=True,
                    )
                dst = yb[:, pair * 2 * C : (pair + 1) * 2 * C]
                if pair < 2:
                    nc.scalar.copy(dst, bank[:])
                else:
                    nc.vector.tensor_copy(dst, bank[:])
                if pair == 1:
                    ydst = y[g].rearrange("p (h c) -> p h c", h=2)[:, 0, :]
                    nc.gpsimd.dma_start(out=ydst, in_=yb[:, : H * C])
                elif pair == 3:
                    if g == NG - 1:
                        # last group: two quarter DMAs to shorten the tail
                        yq = y[g].rearrange("p (q c) -> p q c", q=4)
                        nc.gpsimd.dma_start(
                            out=yq[:, 2, :], in_=yb[:, H * C : 3 * KS * C // 4]
                        )
                        nc.gpsimd.dma_start(
                            out=yq[:, 3, :], in_=yb[:, 3 * KS * C // 4 :]
                        )
                    else:
                        ydst = y[g].rearrange("p (h c) -> p h c", h=2)[:, 1, :]
                        nc.gpsimd.dma_start(out=ydst, in_=yb[:, H * C :])

        # software pipeline, 3-iteration skew: carries(it) | UT(it-3) —
        # the carry chain (~5.5us) fits in the FIFO slack.
        SKEW = 3
        for it in range(NG + SKEW):
            if it < NG:
                carry_stage(it)
            if it >= SKEW:
                ut_stage(it - SKEW)

    nc.compile()
    return nc


def _get_program():
    if "nc" not in _CACHE:
        _CACHE["nc"] = _build_program()
    return _CACHE["nc"]


def _shard(X):
    """(4, 8192, 32, 32) f32 -> 8 fp16 slabs swizzled to (g, p, ks, c)."""
    Xv = X.reshape(B, L, C_FULL)
    shards = []
    for i in range(N_CORES):
        b, h = i // 2, i % 2
        s = Xv[b, :, h * C : (h + 1) * C].astype(np.float16)    # (8192, 512)
        s = s.reshape(NG, KS, P, C).transpose(0, 2, 1, 3)       # (g, p, ks, c)
        shards.append(np.ascontiguousarray(s.reshape(NG, P, KS * C)))
    return shards


def _unshard(parts):
    out = np.empty((B, L, C_FULL), dtype=np.float32)
    for i in range(N_CORES):
        b, h = i // 2, i % 2
        p = parts[i].reshape(NG, P, KS, C).transpose(0, 2, 1, 3)  # (g, ks, p, c)
        out[b, :, h * C : (h + 1) * C] = p.reshape(L, C).astype(np.float32)
    return out.reshape(B, L, D, N)


def kernel(X_in, _trace=False, _tmpdir=None, _trace_cores=None):
    X = np.asarray(X_in, dtype=np.float32)
    assert X.shape == (B, L, D, N), X.shape
    nc = _get_program()
    cst = _host_consts()
    in_maps = [{"x": s, "cst": cst} for s in _shard(X)]
    kwargs = {}
    if _trace:
        kwargs = dict(
            trace=True,
            tmpdir=_tmpdir,
            trace_cores=_trace_cores or list(range(N_CORES)),
        )
    res = run_bass_kernel_spmd(nc, in_maps, core_ids=list(range(N_CORES)), **kwargs)
    out = _unshard([res.results[i]["y"] for i in range(N_CORES)])
    kernel.last_results = res
    return out
